# revision 7
# baseline (speedup 1.0000x reference)
"""Trainium2 kernel for DeepFeatureMatcher (mutual nearest-neighbor matching).

Full inputs: map_A, map_B [1, 512, 96, 128] fp32.
Host normalizes descriptors; 8 NeuronCores each compute one row-slab of the
similarity matrix for each direction (A->B and B->A) plus top-2 (value+index)
per row; host performs the ratio test and mutual check.
"""
import sys

sys.path.insert(0, '/opt/trn_rl_repo')

import numpy as np

CH = 512
N1 = 96 * 128  # 12288
N2 = 96 * 128
N_CORES = 8
SLAB = N1 // N_CORES          # 1536 rows per core
M_TILES = SLAB // 128         # 12
KT = CH // 128                # 4 k-tiles
CB = 2048                     # psum block width (4 banks)
NCB = N2 // CB                # 6 chunk blocks
RATIO = 0.95
EPS = 1e-8

_compiled = None
LAST_EXEC_NS = None
LAST_RESULTS = None


def _build():
    import concourse.bacc as bacc
    import concourse.tile as tile
    from concourse import mybir

    nc = bacc.Bacc("TRN2", target_bir_lowering=False, debug=False,
                   num_devices=N_CORES)

    # inputs: per-direction stationary slab (transposed) + full moving matrix
    lhsT_d = [nc.dram_tensor(f"lhsT{d}", [CH, SLAB], mybir.dt.float32,
                             kind="ExternalInput") for d in range(2)]
    rhs_d = [nc.dram_tensor(f"rhs{d}", [CH, N2], mybir.dt.float32,
                            kind="ExternalInput") for d in range(2)]
    vals_d = [nc.dram_tensor(f"vals{d}", [M_TILES, 128, NCB, 8],
                             mybir.dt.float32, kind="ExternalOutput")
              for d in range(2)]
    idxs_d = [nc.dram_tensor(f"idxs{d}", [M_TILES, 128, NCB, 8],
                             mybir.dt.uint32, kind="ExternalOutput")
              for d in range(2)]

    with tile.TileContext(nc) as tc:
        with tc.tile_pool(name="lhs", bufs=2) as lhs_pool, \
             tc.tile_pool(name="rhs", bufs=3) as rhs_pool, \
             tc.tile_pool(name="psum", bufs=2, space="PSUM") as psum_pool, \
             tc.tile_pool(name="stats", bufs=2) as stats_pool:
            for d in range(2):
                lhsT = lhs_pool.tile([128, KT, SLAB], mybir.dt.float32,
                                     tag="lhsT")
                nc.sync.dma_start(
                    out=lhsT[:],
                    in_=lhsT_d[d].ap().rearrange("(kt p) m -> p kt m", p=128))

                sv = stats_pool.tile([128, M_TILES, NCB, 8],
                                     mybir.dt.float32, tag="sv")
                si = stats_pool.tile([128, M_TILES, NCB, 8],
                                     mybir.dt.uint32, tag="si")

                for cb in range(NCB):
                    rt = rhs_pool.tile([128, KT, CB], mybir.dt.float32,
                                       tag="rhs")
                    nc.sync.dma_start(
                        out=rt[:],
                        in_=rhs_d[d].ap()[:, cb * CB:(cb + 1) * CB]
                        .rearrange("(kt p) n -> p kt n", p=128))
                    for m in range(M_TILES):
                        ps = psum_pool.tile([128, CB], mybir.dt.float32,
                                            tag="ps")
                        for c in range(4):
                            for k in range(KT):
                                nc.tensor.matmul(
                                    out=ps[:, c * 512:(c + 1) * 512],
                                    lhsT=lhsT[:, k, m * 128:(m + 1) * 128],
                                    rhs=rt[:, k, c * 512:(c + 1) * 512],
                                    start=(k == 0), stop=(k == KT - 1))
                        nc.vector.max(sv[:, m, cb], ps[:])
                        nc.vector.max_index(si[:, m, cb], sv[:, m, cb], ps[:])

                nc.sync.dma_start(
                    out=vals_d[d].ap().rearrange("m p c e -> p m c e"),
                    in_=sv[:])
                nc.sync.dma_start(
                    out=idxs_d[d].ap().rearrange("m p c e -> p m c e"),
                    in_=si[:])

    nc.compile()
    return nc


def _get_compiled():
    global _compiled
    if _compiled is None:
        _compiled = _build()
    return _compiled


def _normalize(fmap):
    # [1, ch, h, w] -> [ch, h*w], columns L2-normalized (fp32, matches ref)
    d = fmap.reshape(CH, -1).astype(np.float32)
    nrm = np.sqrt(np.sum(np.square(d), axis=0, keepdims=True,
                         dtype=np.float32))
    return (d / nrm).astype(np.float32)


def _combine(vals, idxs):
    # vals/idxs: [R, NCB, 8] -> per-row top1 val, top1 idx, top2 val
    c1 = vals[:, :, 0]
    c2 = vals[:, :, 1]
    j = np.argmax(c1, axis=1)
    r = np.arange(c1.shape[0])
    m1 = c1[r, j]
    i1 = idxs[r, j, 0].astype(np.int64) + j * CB
    c1m = c1.copy()
    c1m[r, j] = -np.inf
    s = c1m.max(axis=1)
    m2 = np.maximum(s, c2[r, j])
    return m1, i1, m2


def _install_trace_shim():
    # this image lacks antenv.axon_hooks; recreate it from the boot module
    # so run_bass_kernel_spmd(trace=True) can NTFF-profile, and stub the
    # artifact upload (no bucket access here).
    import types

    try:
        import antenv.axon_hooks  # noqa: F401
    except ImportError:
        from trn_agent_boot.trn_boot import _ntff_profile_via_ctypes
        hook = _ntff_profile_via_ctypes('/opt/axon/libaxon_pjrt.so')
        mod = types.ModuleType('antenv.axon_hooks')
        mod.get_axon_ntff_profile_hook = lambda: hook
        mod.set_axon_ntff_profile_hook = lambda h: None
        sys.modules['antenv.axon_hooks'] = mod
    import concourse.bass_utils as bu
    bu.upload_artifacts = lambda tmpdir: tmpdir


def kernel(map_A, map_B):
    import os

    from concourse.bass_utils import run_bass_kernel_spmd

    global LAST_EXEC_NS, LAST_RESULTS
    trace = bool(int(os.environ.get("KERNEL_TRACE", "0")))
    if trace:
        _install_trace_shim()
    nc = _get_compiled()

    nA = _normalize(np.asarray(map_A))  # [512, 12288] columns normalized
    nB = _normalize(np.asarray(map_B))

    in_maps = []
    for c in range(N_CORES):
        sl = slice(c * SLAB, (c + 1) * SLAB)
        in_maps.append({
            "lhsT0": np.ascontiguousarray(nA[:, sl]),
            "rhs0": nB,
            "lhsT1": np.ascontiguousarray(nB[:, sl]),
            "rhs1": nA,
        })

    res = run_bass_kernel_spmd(nc, in_maps, core_ids=list(range(N_CORES)),
                               trace=trace)
    LAST_EXEC_NS = res.exec_time_ns
    LAST_RESULTS = res

    m1 = [None, None]
    i1 = [None, None]
    m2 = [None, None]
    for d in range(2):
        mv, mi, ms = [], [], []
        for c in range(N_CORES):
            v = res.results[c][f"vals{d}"].reshape(SLAB, NCB, 8)
            ix = res.results[c][f"idxs{d}"].reshape(SLAB, NCB, 8)
            a, b, e = _combine(v, ix)
            mv.append(a)
            mi.append(b)
            ms.append(e)
        m1[d] = np.concatenate(mv)
        i1[d] = np.concatenate(mi)
        m2[d] = np.concatenate(ms)

    match_sim = m1[0]
    nn12 = i1[0]
    dist12_1 = 2.0 - 2.0 * m1[0]
    dist12_2 = 2.0 - 2.0 * m2[0]
    ratios12 = dist12_1 / (dist12_2 + EPS)
    nn21 = i1[1]
    dist21_1 = 2.0 - 2.0 * m1[1]
    dist21_2 = 2.0 - 2.0 * m2[1]
    ratios21 = dist21_1 / (dist21_2 + EPS)

    ids1 = np.arange(N1)
    mask = ((ids1 == nn21[nn12]) & (ratios12 <= RATIO)
            & (ratios21[nn12] <= RATIO))
    masked_sim = np.where(mask, match_sim, 0.0).astype(np.float32)
    return masked_sim, nn12.astype(np.int32), mask


# revision 12
# speedup vs baseline: 1.8860x; 1.8860x over previous
"""Trainium2 kernel for DeepFeatureMatcher (mutual nearest-neighbor matching).

Full inputs: map_A, map_B [1, 512, 96, 128] fp32.
Host normalizes descriptors; 8 NeuronCores each compute one row-slab of the
similarity matrix for each direction (A->B and B->A) plus top-2 (value+index)
per row; host performs the ratio test and mutual check.
"""
import sys

sys.path.insert(0, '/opt/trn_rl_repo')

import numpy as np

CH = 512
N1 = 96 * 128  # 12288
N2 = 96 * 128
N_CORES = 8
SLAB = N1 // N_CORES          # 1536 rows per core
M_TILES = SLAB // 128         # 12
KT = CH // 128                # 4 k-tiles
CB = 2048                     # psum block width (4 banks)
NCB = N2 // CB                # 6 chunk blocks
RATIO = 0.95
EPS = 1e-8

_compiled = None
LAST_EXEC_NS = None
LAST_RESULTS = None


def _build():
    import concourse.bacc as bacc
    import concourse.tile as tile
    from concourse import mybir

    nc = bacc.Bacc("TRN2", target_bir_lowering=False, debug=False,
                   num_devices=N_CORES)

    # inputs: per-direction stationary slab (transposed) + full moving matrix,
    # each split into hi (11-bit mantissa, exact in float32r) + lo residual.
    # 3 f32r matmuls (hh, hl, lh) reproduce the fp32 product to ~2^-24.
    lhsT_d = [[nc.dram_tensor(f"lhsT{d}{p}", [CH, SLAB], mybir.dt.float32r,
                              kind="ExternalInput") for p in "hl"]
              for d in range(2)]
    rhs_d = [[nc.dram_tensor(f"rhs{d}{p}", [CH, N2], mybir.dt.float32r,
                             kind="ExternalInput") for p in "hl"]
             for d in range(2)]
    vals_d = [nc.dram_tensor(f"vals{d}", [M_TILES, 128, NCB, 8],
                             mybir.dt.float32, kind="ExternalOutput")
              for d in range(2)]
    idxs_d = [nc.dram_tensor(f"idxs{d}", [M_TILES, 128, NCB, 8],
                             mybir.dt.uint32, kind="ExternalOutput")
              for d in range(2)]

    with tile.TileContext(nc) as tc:
        with tc.tile_pool(name="lhs", bufs=1) as lhs_pool, \
             tc.tile_pool(name="rhs", bufs=2) as rhs_pool, \
             tc.tile_pool(name="psum", bufs=2, space="PSUM") as psum_pool, \
             tc.tile_pool(name="stats", bufs=1) as stats_pool:
            for d in range(2):
                lh = lhs_pool.tile([128, KT, SLAB], mybir.dt.float32r,
                                   tag="lh")
                ll = lhs_pool.tile([128, KT, SLAB], mybir.dt.float32r,
                                   tag="ll")
                for t, src in ((lh, lhsT_d[d][0]), (ll, lhsT_d[d][1])):
                    nc.sync.dma_start(
                        out=t[:],
                        in_=src.ap().rearrange("(kt p) m -> p kt m", p=128))

                sv = stats_pool.tile([128, M_TILES, NCB, 8],
                                     mybir.dt.float32, tag="sv")
                si = stats_pool.tile([128, M_TILES, NCB, 8],
                                     mybir.dt.uint32, tag="si")

                for cb in range(NCB):
                    rh = rhs_pool.tile([128, KT, CB], mybir.dt.float32r,
                                       tag="rh")
                    rl = rhs_pool.tile([128, KT, CB], mybir.dt.float32r,
                                       tag="rl")
                    for t, src in ((rh, rhs_d[d][0]), (rl, rhs_d[d][1])):
                        nc.sync.dma_start(
                            out=t[:],
                            in_=src.ap()[:, cb * CB:(cb + 1) * CB]
                            .rearrange("(kt p) n -> p kt n", p=128))
                    for m in range(M_TILES):
                        ps = psum_pool.tile([128, CB], mybir.dt.float32,
                                            tag="ps")
                        msl = slice(m * 128, (m + 1) * 128)
                        n_mm = KT * 3
                        for k in range(KT):
                            # same stationary operand across all 4 column
                            # slices -> one weight load per (k, pair)
                            for i, (lt, rt) in enumerate(
                                    ((lh, rh), (lh, rl), (ll, rh))):
                                for c in range(4):
                                    nc.tensor.matmul(
                                        out=ps[:, c * 512:(c + 1) * 512],
                                        lhsT=lt[:, k, msl],
                                        rhs=rt[:, k, c * 512:(c + 1) * 512],
                                        start=(k == 0 and i == 0),
                                        stop=(k == KT - 1 and i == 2))
                        nc.vector.max(sv[:, m, cb], ps[:])
                        nc.vector.max_index(si[:, m, cb], sv[:, m, cb], ps[:])

                nc.sync.dma_start(
                    out=vals_d[d].ap().rearrange("m p c e -> p m c e"),
                    in_=sv[:])
                nc.sync.dma_start(
                    out=idxs_d[d].ap().rearrange("m p c e -> p m c e"),
                    in_=si[:])

    nc.compile()
    return nc


def _get_compiled():
    global _compiled
    if _compiled is None:
        _compiled = _build()
    return _compiled


def _split_hi_lo(x):
    # hi = x rounded to 11 explicit mantissa bits (exactly representable in
    # float32r); lo = exact fp32 residual (<= 12 significant bits).
    u = x.view(np.uint32)
    r = ((u + np.uint32(1 << 11)) & np.uint32(0xFFFFF000)).view(np.float32)
    return r, (x - r).astype(np.float32)


def _normalize(fmap):
    # [1, ch, h, w] -> [ch, h*w], columns L2-normalized (fp32, matches ref)
    d = fmap.reshape(CH, -1).astype(np.float32)
    nrm = np.sqrt(np.sum(np.square(d), axis=0, keepdims=True,
                         dtype=np.float32))
    return (d / nrm).astype(np.float32)


def _combine(vals, idxs):
    # vals/idxs: [R, NCB, 8] -> per-row top1 val, top1 idx, top2 val
    c1 = vals[:, :, 0]
    c2 = vals[:, :, 1]
    j = np.argmax(c1, axis=1)
    r = np.arange(c1.shape[0])
    m1 = c1[r, j]
    i1 = idxs[r, j, 0].astype(np.int64) + j * CB
    c1m = c1.copy()
    c1m[r, j] = -np.inf
    s = c1m.max(axis=1)
    m2 = np.maximum(s, c2[r, j])
    return m1, i1, m2


def _install_trace_shim():
    # this image lacks antenv.axon_hooks; recreate it from the boot module
    # so run_bass_kernel_spmd(trace=True) can NTFF-profile, and stub the
    # artifact upload (no bucket access here).
    import types

    try:
        import antenv.axon_hooks  # noqa: F401
    except ImportError:
        from trn_agent_boot.trn_boot import _ntff_profile_via_ctypes
        hook = _ntff_profile_via_ctypes('/opt/axon/libaxon_pjrt.so')
        mod = types.ModuleType('antenv.axon_hooks')
        mod.get_axon_ntff_profile_hook = lambda: hook
        mod.set_axon_ntff_profile_hook = lambda h: None
        sys.modules['antenv.axon_hooks'] = mod
    import concourse.bass_utils as bu
    bu.upload_artifacts = lambda tmpdir: tmpdir


def kernel(map_A, map_B):
    import os

    from concourse.bass_utils import run_bass_kernel_spmd

    global LAST_EXEC_NS, LAST_RESULTS
    trace = bool(int(os.environ.get("KERNEL_TRACE", "0")))
    if trace:
        _install_trace_shim()
    nc = _get_compiled()

    nA = _normalize(np.asarray(map_A))  # [512, 12288] columns normalized
    nB = _normalize(np.asarray(map_B))
    nAh, nAl = _split_hi_lo(nA)
    nBh, nBl = _split_hi_lo(nB)

    in_maps = []
    for c in range(N_CORES):
        sl = slice(c * SLAB, (c + 1) * SLAB)
        in_maps.append({
            "lhsT0h": np.ascontiguousarray(nAh[:, sl]),
            "lhsT0l": np.ascontiguousarray(nAl[:, sl]),
            "rhs0h": nBh,
            "rhs0l": nBl,
            "lhsT1h": np.ascontiguousarray(nBh[:, sl]),
            "lhsT1l": np.ascontiguousarray(nBl[:, sl]),
            "rhs1h": nAh,
            "rhs1l": nAl,
        })

    res = run_bass_kernel_spmd(nc, in_maps, core_ids=list(range(N_CORES)),
                               trace=trace)
    LAST_EXEC_NS = res.exec_time_ns
    LAST_RESULTS = res

    m1 = [None, None]
    i1 = [None, None]
    m2 = [None, None]
    for d in range(2):
        mv, mi, ms = [], [], []
        for c in range(N_CORES):
            v = res.results[c][f"vals{d}"].reshape(SLAB, NCB, 8)
            ix = res.results[c][f"idxs{d}"].reshape(SLAB, NCB, 8)
            a, b, e = _combine(v, ix)
            mv.append(a)
            mi.append(b)
            ms.append(e)
        m1[d] = np.concatenate(mv)
        i1[d] = np.concatenate(mi)
        m2[d] = np.concatenate(ms)

    match_sim = m1[0]
    nn12 = i1[0]
    dist12_1 = 2.0 - 2.0 * m1[0]
    dist12_2 = 2.0 - 2.0 * m2[0]
    ratios12 = dist12_1 / (dist12_2 + EPS)
    nn21 = i1[1]
    dist21_1 = 2.0 - 2.0 * m1[1]
    dist21_2 = 2.0 - 2.0 * m2[1]
    ratios21 = dist21_1 / (dist21_2 + EPS)

    ids1 = np.arange(N1)
    mask = ((ids1 == nn21[nn12]) & (ratios12 <= RATIO)
            & (ratios21[nn12] <= RATIO))
    masked_sim = np.where(mask, match_sim, 0.0).astype(np.float32)
    return masked_sim, nn12.astype(np.int32), mask


# revision 13
# speedup vs baseline: 1.9455x; 1.0316x over previous
"""v2: single-matmul + PE-transpose scheme, float32r x3.

Each core computes its A-row slab sim [1536, 12288] once (f32r hi/lo x3
matmuls). Direction A->B reduces rows on VectorE directly; direction B->A
is obtained by transposing each sim tile on TensorE (exact, fp32) and
reducing the transposed column strips; per-core partial column top-2s are
combined across cores on the host.
"""
import sys

sys.path.insert(0, '/opt/trn_rl_repo')

import numpy as np

CH = 512
N1 = 96 * 128
N2 = 96 * 128
N_CORES = 8
SLAB = N1 // N_CORES          # 1536
M_TILES = SLAB // 128         # 12
KT = CH // 128                # 4
CB = 1024                     # matmul block width (2 psum banks)
NCB = N2 // CB                # 12
RATIO = 0.95
EPS = 1e-8

_compiled = None
LAST_EXEC_NS = None
LAST_RESULTS = None


def _build():
    import concourse.bacc as bacc
    import concourse.tile as tile
    from concourse import mybir

    nc = bacc.Bacc("TRN2", target_bir_lowering=False, debug=False,
                   num_devices=N_CORES)

    lhsT_d = [nc.dram_tensor(f"lhsT{p}", [CH, SLAB], mybir.dt.float32r,
                             kind="ExternalInput") for p in "hl"]
    rhs_d = [nc.dram_tensor(f"rhs{p}", [CH, N2], mybir.dt.float32r,
                            kind="ExternalInput") for p in "hl"]
    vals1_d = nc.dram_tensor("vals1", [M_TILES, 128, NCB, 8],
                             mybir.dt.float32, kind="ExternalOutput")
    idxs1_d = nc.dram_tensor("idxs1", [M_TILES, 128, NCB, 8],
                             mybir.dt.uint32, kind="ExternalOutput")
    vals2_d = nc.dram_tensor("vals2", [NCB, 8, 128, 8],
                             mybir.dt.float32, kind="ExternalOutput")
    idxs2_d = nc.dram_tensor("idxs2", [NCB, 8, 128, 8],
                             mybir.dt.uint32, kind="ExternalOutput")

    with tile.TileContext(nc) as tc:
        with tc.tile_pool(name="lhs", bufs=1) as lhs_pool, \
             tc.tile_pool(name="rhs", bufs=2) as rhs_pool, \
             tc.tile_pool(name="sim", bufs=3) as sim_pool, \
             tc.tile_pool(name="strips", bufs=1) as strip_pool, \
             tc.tile_pool(name="psmm", bufs=2, space="PSUM") as psmm_pool, \
             tc.tile_pool(name="pstr", bufs=3, space="PSUM") as pstr_pool, \
             tc.tile_pool(name="stats", bufs=1) as stats_pool:
            lh = lhs_pool.tile([128, KT, SLAB], mybir.dt.float32r, tag="lh")
            ll = lhs_pool.tile([128, KT, SLAB], mybir.dt.float32r, tag="ll")
            for t, src in ((lh, lhsT_d[0]), (ll, lhsT_d[1])):
                nc.sync.dma_start(
                    out=t[:],
                    in_=src.ap().rearrange("(kt p) m -> p kt m", p=128))
            ident_d = nc.inline_tensor(np.eye(128, dtype=np.float32),
                                       name="ident")
            ident = lhs_pool.tile([128, 128], mybir.dt.float32, tag="ident")
            nc.sync.dma_start(out=ident[:], in_=ident_d.ap())

            sv1 = stats_pool.tile([128, M_TILES, NCB, 8], mybir.dt.float32,
                                  tag="sv1")
            si1 = stats_pool.tile([128, M_TILES, NCB, 8], mybir.dt.uint32,
                                  tag="si1")
            sv2 = stats_pool.tile([128, NCB, 8, 8], mybir.dt.float32,
                                  tag="sv2")
            si2 = stats_pool.tile([128, NCB, 8, 8], mybir.dt.uint32,
                                  tag="si2")

            for cb in range(NCB):
                rh = rhs_pool.tile([128, KT, CB], mybir.dt.float32r, tag="rh")
                rl = rhs_pool.tile([128, KT, CB], mybir.dt.float32r, tag="rl")
                for t, src in ((rh, rhs_d[0]), (rl, rhs_d[1])):
                    nc.sync.dma_start(
                        out=t[:],
                        in_=src.ap()[:, cb * CB:(cb + 1) * CB]
                        .rearrange("(kt p) n -> p kt n", p=128))

                # strips[:, b, :] holds cols cb*CB + b*128 .. +128 (partition
                # = col within block), rows = the slab's 1536 A-rows.
                strips = strip_pool.tile([128, 8, SLAB], mybir.dt.float32,
                                         tag="strips")

                for m in range(M_TILES):
                    ps = psmm_pool.tile([128, CB], mybir.dt.float32, tag="ps")
                    msl = slice(m * 128, (m + 1) * 128)
                    for k in range(KT):
                        for i, (lt, rt) in enumerate(
                                ((lh, rh), (lh, rl), (ll, rh))):
                            for c in range(CB // 512):
                                nc.tensor.matmul(
                                    out=ps[:, c * 512:(c + 1) * 512],
                                    lhsT=lt[:, k, msl],
                                    rhs=rt[:, k, c * 512:(c + 1) * 512],
                                    start=(k == 0 and i == 0),
                                    stop=(k == KT - 1 and i == 2))
                    s = sim_pool.tile([128, CB], mybir.dt.float32, tag="s")
                    nc.scalar.copy(s[:], ps[:])
                    # direction 1: rows are A points
                    nc.vector.max(sv1[:, m, cb], s[:])
                    nc.vector.max_index(si1[:, m, cb], sv1[:, m, cb], s[:])
                    # transpose the 8 [128,128] blocks, 4 per psum tile
                    for hb in range(2):
                        pt = pstr_pool.tile([128, 4, 128], mybir.dt.float32,
                                            tag="pt")
                        for j in range(4):
                            b = hb * 4 + j
                            nc.tensor.transpose(
                                pt[:, j], s[:, b * 128:(b + 1) * 128],
                                ident[:])
                        nc.scalar.copy(
                            strips[:, hb * 4:(hb + 1) * 4, msl], pt[:])

                for b in range(8):
                    nc.vector.max(sv2[:, cb, b], strips[:, b])
                    nc.vector.max_index(si2[:, cb, b], sv2[:, cb, b],
                                        strips[:, b])

            nc.sync.dma_start(
                out=vals1_d.ap().rearrange("m p c e -> p m c e"), in_=sv1[:])
            nc.sync.dma_start(
                out=idxs1_d.ap().rearrange("m p c e -> p m c e"), in_=si1[:])
            nc.sync.dma_start(
                out=vals2_d.ap().rearrange("c b p e -> p c b e"), in_=sv2[:])
            nc.sync.dma_start(
                out=idxs2_d.ap().rearrange("c b p e -> p c b e"), in_=si2[:])

    nc.compile()
    return nc


def _get_compiled():
    global _compiled
    if _compiled is None:
        _compiled = _build()
    return _compiled


def _split_hi_lo(x):
    u = x.view(np.uint32)
    r = ((u + np.uint32(1 << 11)) & np.uint32(0xFFFFF000)).view(np.float32)
    return r, (x - r).astype(np.float32)


def _normalize(fmap):
    d = fmap.reshape(CH, -1).astype(np.float32)
    nrm = np.sqrt(np.sum(np.square(d), axis=0, keepdims=True,
                         dtype=np.float32))
    return (d / nrm).astype(np.float32)


def _combine(vals, idxs):
    """vals/idxs: [R, C, 8] chunk top-8s with idxs already global.
    Returns per-row top1 val, top1 idx, top2 val across all chunks."""
    c1 = vals[:, :, 0]
    c2 = vals[:, :, 1]
    j = np.argmax(c1, axis=1)
    r = np.arange(c1.shape[0])
    m1 = c1[r, j]
    i1 = idxs[r, j, 0].astype(np.int64)
    c1m = c1.copy()
    c1m[r, j] = -np.inf
    s = c1m.max(axis=1)
    m2 = np.maximum(s, c2[r, j])
    return m1, i1, m2


def _install_trace_shim():
    import types

    try:
        import antenv.axon_hooks  # noqa: F401
    except ImportError:
        from trn_agent_boot.trn_boot import _ntff_profile_via_ctypes
        hook = _ntff_profile_via_ctypes('/opt/axon/libaxon_pjrt.so')
        mod = types.ModuleType('antenv.axon_hooks')
        mod.get_axon_ntff_profile_hook = lambda: hook
        mod.set_axon_ntff_profile_hook = lambda h: None
        sys.modules['antenv.axon_hooks'] = mod
    import concourse.bass_utils as bu
    bu.upload_artifacts = lambda tmpdir: tmpdir


def kernel(map_A, map_B):
    import os

    from concourse.bass_utils import run_bass_kernel_spmd

    global LAST_EXEC_NS, LAST_RESULTS
    trace = bool(int(os.environ.get("KERNEL_TRACE", "0")))
    if trace:
        _install_trace_shim()
    nc = _get_compiled()

    nA = _normalize(np.asarray(map_A))
    nB = _normalize(np.asarray(map_B))
    nAh, nAl = _split_hi_lo(nA)
    nBh, nBl = _split_hi_lo(nB)

    in_maps = []
    for c in range(N_CORES):
        sl = slice(c * SLAB, (c + 1) * SLAB)
        in_maps.append({
            "lhsTh": np.ascontiguousarray(nAh[:, sl]),
            "lhsTl": np.ascontiguousarray(nAl[:, sl]),
            "rhsh": nBh,
            "rhsl": nBl,
        })

    res = run_bass_kernel_spmd(nc, in_maps, core_ids=list(range(N_CORES)),
                               trace=trace)
    LAST_EXEC_NS = res.exec_time_ns
    LAST_RESULTS = res

    # direction 1: concatenate row slabs; chunk idx offset = cb*CB
    mv, mi, ms = [], [], []
    off = (np.arange(NCB, dtype=np.int64) * CB)[None, :, None]
    for c in range(N_CORES):
        v = res.results[c]["vals1"].reshape(SLAB, NCB, 8)
        ix = res.results[c]["idxs1"].reshape(SLAB, NCB, 8).astype(np.int64)
        a, b, e = _combine(v, ix + off)
        mv.append(a)
        mi.append(b)
        ms.append(e)
    m1_12 = np.concatenate(mv)
    nn12 = np.concatenate(mi)
    m2_12 = np.concatenate(ms)

    # direction 2: per-core partial top-8 over its slab rows; combine cores
    v2 = np.stack([res.results[c]["vals2"].reshape(N2, 8)
                   for c in range(N_CORES)], axis=1)  # [N2, cores, 8]
    i2 = np.stack([res.results[c]["idxs2"].reshape(N2, 8).astype(np.int64)
                   + c * SLAB for c in range(N_CORES)], axis=1)
    m1_21, nn21, m2_21 = _combine(v2, i2)

    match_sim = m1_12
    ratios12 = (2.0 - 2.0 * m1_12) / ((2.0 - 2.0 * m2_12) + EPS)
    ratios21 = (2.0 - 2.0 * m1_21) / ((2.0 - 2.0 * m2_21) + EPS)

    ids1 = np.arange(N1)
    mask = ((ids1 == nn21[nn12]) & (ratios12 <= RATIO)
            & (ratios21[nn12] <= RATIO))
    masked_sim = np.where(mask, match_sim, 0.0).astype(np.float32)
    return masked_sim, nn12.astype(np.int32), mask
